# revision 5
# baseline (speedup 1.0000x reference)
"""KoLeo loss kernel for Trainium2 (8 NeuronCores, SPMD row-sharded).

Algorithm (matches the jax reference):
  feats_n = features / ||features||_row          (L2 row normalize)
  C       = feats_n @ feats_n.T                  (cosine similarity, NxN)
  m_i     = max_{j != i} C[i, j]                 (nearest-neighbor cosine)
  dist_i  = sqrt(2 - 2 m_i)                      (= ||f_i - f_j*|| for unit vectors)
  loss    = -mean(log(dist_i + 1e-8))

Sharding: each of the 8 cores gets the FULL features plus its 2048-row shard;
it computes cosine rows (shard x N) in bf16 on the TensorEngine and reduces a
per-row max (diagonal suppressed via a data-driven -3*I mask so that the SPMD
program is identical across cores). The tiny final sqrt/log/mean runs on host.

Device pipeline per core:
  - stream B row-tiles [128, D] fp32, compute row norms on ScalarE
    (Square + accum), rsqrt via DVE reciprocal + ScalarE sqrt,
    scale+cast to bf16 on DVE, transpose via TensorE into B^T layout.
  - matmul: for each (i, n) output tile [128, 512], accumulate 8 k-chunks
    in PSUM (bf16 inputs, fp32 accumulate).
  - per-chunk row-max on DVE straight from PSUM; diagonal chunk uses the
    fused tensor_tensor_reduce(add, max) with a sliding -3*eye window.
"""

import numpy as np

P = 128  # SBUF partitions
NCH = 512  # output chunk columns (one PSUM bank of fp32)

N_FULL = 16384
D_FULL = 1024
NCORES = 8


def _build(N, D, NC):
    import concourse.bacc as bacc
    import concourse.mybir as mybir
    from concourse import masks, tile

    f32 = mybir.dt.float32
    bf16 = mybir.dt.bfloat16
    AF = mybir.ActivationFunctionType

    SH = N // NC  # shard rows per core
    JB = SH  # column-block size (must equal SH so diag block index == core id)
    nJ = N // JB  # column blocks
    nI = SH // P  # row tiles in shard
    nK = D // P  # contraction chunks
    nR = JB // P  # row tiles per column block
    nch = min(NCH, JB)  # reduce-chunk width
    nN = JB // nch  # chunks per column block
    nPP = nch // P  # diag positions per chunk
    ncht = nJ * nN  # chunks per output row

    nc = bacc.Bacc("TRN2", target_bir_lowering=False, debug=False)
    feat = nc.dram_tensor("features", [N, D], f32, kind="ExternalInput").ap()
    ash = nc.dram_tensor("ashard", [SH, D], f32, kind="ExternalInput").ap()
    oh_d = nc.dram_tensor("onehot", [P, nJ], f32, kind="ExternalInput").ap()
    outd = nc.dram_tensor("maxcos", [SH], f32, kind="ExternalOutput").ap()

    with tile.TileContext(nc) as tc:
        with (
            tc.tile_pool(name="const", bufs=1) as constp,
            tc.tile_pool(name="at", bufs=1) as atp,
            tc.tile_pool(name="bt", bufs=2) as btp,
            tc.tile_pool(name="x", bufs=4) as xp,
            tc.tile_pool(name="xn", bufs=3) as xnp,
            tc.tile_pool(name="sq", bufs=2) as sqp,
            tc.tile_pool(name="s", bufs=4) as sp_,
            tc.tile_pool(name="z3", bufs=2) as z3p,
            tc.tile_pool(name="macc", bufs=1) as maccp,
            tc.tile_pool(name="fin", bufs=1) as finp,
            tc.tile_pool(name="pmm", bufs=4, space="PSUM") as pmm,
            tc.tile_pool(name="ptp", bufs=3, space="PSUM") as ptp,
        ):
            ident = constp.tile([P, P], bf16)
            masks.make_identity(nc, ident[:])
            eyef = constp.tile([P, P], f32)
            masks.make_identity(nc, eyef[:])
            oh = constp.tile([P, nJ], f32)
            nc.sync.dma_start(out=oh[:], in_=oh_d)
            maxacc = maccp.tile([P, nI * ncht], f32)
            fin = finp.tile([P, nI], f32)

            def prep(src, dst, nrt):
                # src: DRAM [nrt*P, D] fp32 -> dst: SBUF [P, nK*nrt*P] bf16,
                # normalized and transposed (dst[:, k*nrt*P + g] = row g of
                # d-chunk k).
                dstv = dst.rearrange("p (k c) -> p k c", k=nK)
                for r in range(nrt):
                    x = xp.tile([P, D], f32)
                    nc.sync.dma_start(out=x[:], in_=src[r * P : (r + 1) * P, :])
                    sq = sqp.tile([P, D], f32)
                    ssq = sp_.tile([P, 1], f32)
                    nc.scalar.activation(sq[:], x[:], AF.Square, accum_out=ssq[:])
                    rs = sp_.tile([P, 1], f32)
                    nc.vector.reciprocal(rs[:], ssq[:])
                    s2 = sp_.tile([P, 1], f32)
                    nc.scalar.activation(s2[:], rs[:], AF.Sqrt)
                    xn = xnp.tile([P, D], bf16)
                    nc.vector.tensor_scalar_mul(xn[:], x[:], s2[:])
                    tp = ptp.tile([P, nK * P], bf16)
                    tpv = tp.rearrange("p (k c) -> p k c", k=nK)
                    for k in range(nK):
                        nc.tensor.transpose(
                            tpv[:, k, :], xn[:, k * P : (k + 1) * P], ident[:]
                        )
                    nc.scalar.copy(dstv[:, :, r * P : (r + 1) * P], tpv[:, :, :])

            at = atp.tile([P, nK * SH], bf16)
            prep(ash, at, nI)

            live = {}

            def prep_b(j):
                bt = btp.tile([P, nK * JB], bf16)
                prep(feat[j * JB : (j + 1) * JB, :], bt, nR)
                seye = z3p.tile([P, P], f32)
                nc.vector.tensor_scalar_mul(seye[:], eyef[:], oh[:, j : j + 1])
                live[j] = (bt, seye)

            prep_b(0)
            for j in range(nJ):
                if j + 1 < nJ:
                    prep_b(j + 1)  # emitted early so prep overlaps matmuls
                bt, seye = live.pop(j)
                for i in range(nI):
                    for n in range(nN):
                        ps = pmm.tile([P, nch], f32)
                        for k in range(nK):
                            nc.tensor.matmul(
                                ps[:],
                                at[:, k * SH + i * P : k * SH + (i + 1) * P],
                                bt[:, k * JB + n * nch : k * JB + (n + 1) * nch],
                                start=(k == 0),
                                stop=(k == nK - 1),
                            )
                        slot = i * ncht + j * nN + n
                        if n == (i * P) // nch:
                            # chunk holding this i-tile's diagonal when j == c:
                            # add -3*eye (zeros when j != c) in place, then max
                            pp = i % nPP
                            nc.vector.tensor_add(
                                ps[:, pp * P : (pp + 1) * P],
                                ps[:, pp * P : (pp + 1) * P],
                                seye[:],
                            )
                        nc.vector.reduce_max(
                            maxacc[:, slot : slot + 1],
                            ps[:],
                            axis=mybir.AxisListType.X,
                        )
            for i in range(nI):
                nc.vector.reduce_max(
                    fin[:, i : i + 1],
                    maxacc[:, i * ncht : (i + 1) * ncht],
                    axis=mybir.AxisListType.X,
                )
            nc.sync.dma_start(out=outd.rearrange("(i p) -> p i", p=P), in_=fin[:])

    nc.compile()
    return nc


_CACHE = {}


def _get_nc(N, D, NC):
    key = (N, D, NC)
    if key not in _CACHE:
        _CACHE[key] = _build(N, D, NC)
    return _CACHE[key]


def _in_maps(feats, NC):
    SH = feats.shape[0] // NC
    maps = []
    for c in range(NC):
        oh = np.zeros((P, NC), np.float32)
        oh[:, c] = -3.0
        maps.append(
            {
                "features": feats,
                "ashard": np.ascontiguousarray(feats[c * SH : (c + 1) * SH]),
                "onehot": oh,
            }
        )
    return maps


def _loss_from_maxcos(m):
    dist = np.sqrt(np.maximum(2.0 - 2.0 * m.astype(np.float64), 0.0))
    return np.asarray(-np.mean(np.log(dist + 1e-8)), dtype=np.float32)


def kernel(features):
    from concourse.bass_utils import run_bass_kernel_spmd

    feats = np.ascontiguousarray(np.asarray(features, dtype=np.float32))
    N, D = feats.shape
    nc = _get_nc(N, D, NCORES)
    res = run_bass_kernel_spmd(nc, _in_maps(feats, NCORES), list(range(NCORES)))
    m = np.concatenate([res.results[c]["maxcos"] for c in range(NCORES)])
    return _loss_from_maxcos(m)
